# revision 27
# baseline (speedup 1.0000x reference)
"""Trainium2 Bass kernel for nn_ErdosLoss (graph loss function).

Math (reference reformulated, validated to ~1e-6 rel err):
  penalty:  log_score = scatter_add(log(1 - p + 1e-6), tgt)   over N nodes
            loss2 = mean(exp(log_score)) * 9600
  loss3:    p @ triu(H H^T, 1) @ p^T  ==  (||s||^2 - sum_e d_e p_e^2) / 2
            where s = scatter_add(p, tgt) + scatter_add(p, src | src != tgt),
            d_e = 2 - m_e, m_e = (src_e == tgt_e).
  out = loss2 + 200 * loss3 / num_graphs,  num_graphs = max(batch) + 1.

Single-launch single-core design (per-NEFF fixed overhead here is ~11us, so
any second launch loses):
  Host (index-only preprocessing; values are only reordered, never
  combined): sort the (node, value) scatter pairs by node, bin them so that
  partition p holds exactly nodes [32p, 32(p+1)), and emit aligned [128, K]
  arrays per list: V (values), RM (0 at each node's first pair, else 1),
  LM (1 at each node's last pair, else 0).

  Device: both lists ride ONE segment scan.  A single f16 tile BIG holds
  [Ln-region(K1) | V2(K2) | RM(K1+K2) | LM(K1+K2)]: ACT's Ln writes the
  left K1 columns in place, one f16 DMA fills the rest, so the combined
  tensor_tensor_scan (state = RM*state + v, resetting at segment starts)
  covers the log list and the s list back to back (each partition's s
  section starts with rm=0, so no cross-contamination).  d = r*LM isolates
  node totals; exp over the log half row-accumulates (masked slots count
  exp(0)=1 -> compile-time correction 128*K1-4000; empty nodes cancel);
  the s half yields sum(d^2) - sum(V2^2) via one accumulated product and
  an ACT Square.  R = [SC*er - SC*corr/128 | dsq]; ones-matmul partition
  reduce to PSUM [1,2]; res = F1*(100/num_graphs) + F0; one [1,1] DMA out.

  Latency layout: one explicit ACT table load (set 6 = Ln+Exp+Square+Copy,
  so no mid-kernel reloads); exactly two f16 input transfers (V1+misc on
  sync, the [V2|RM|LM] block on scalar) since a queue's second transfer
  pays a ~0.7us re-arm gap; the exp mask-mult is split off first so ACT's
  Exp overlaps the s-half mask-mult; num_graphs is host-replicated across
  partitions so 100/num_graphs folds into the per-partition accumulations
  and the ones-matmul emits the final scalar directly.
"""

import numpy as np

import concourse.bacc as bacc
import concourse.mybir as mybir
import concourse.tile as tile
from concourse import bass_utils

F32 = mybir.dt.float32
F16 = mybir.dt.float16
ALU = mybir.AluOpType
ACT = mybir.ActivationFunctionType

N_NODES = 4000
PENALTY_SCALE = 16 * 200 * 3   # 9600
SC = PENALTY_SCALE / N_NODES   # 2.4
NPP = 32                       # nodes per partition (128 * 32 = 4096 >= 4000)

K1 = 64    # log-list slots per partition  (measured max 63)
K2 = 120   # s-list slots per partition    (measured max 120; exact fit --
           # _pack_list falls back to a wider rebuild if ever exceeded)


def _build(k1: int, k2: int):
    nc = bacc.Bacc("TRN2", target_bir_lowering=False, debug=False, num_devices=1)

    w = k1 + k2
    dv = nc.dram_tensor("dinv", [128, k1 + 1], F16, kind="ExternalInput").ap()
    df1 = nc.dram_tensor("dinf1", [128, k2 + w], F16, kind="ExternalInput").ap()
    df2 = nc.dram_tensor("dinf2", [128, w], F16, kind="ExternalInput").ap()
    outd = nc.dram_tensor("out", [1, 1], F32, kind="ExternalOutput").ap()

    with tile.TileContext(nc) as tc:
        with (
            tc.tile_pool(name="pool", bufs=1) as pool,
            tc.tile_pool(name="psum", bufs=1, space="PSUM") as ppool,
        ):
            # one ACT table set covering Ln+Exp+Square+Copy (set 6); the
            # auto placement pass then inserts no further loads
            nc.scalar.add_instruction(mybir.InstLoadActFuncSet(
                name="actload6", ins=[], outs=[], act_func_set_id=6))

            # two input transfers, one per HWDGE queue (a second transfer on
            # the same queue pays a ~0.7us DGE re-arm gap)
            Bv = pool.tile([128, k1 + 1], F16, tag="Bv")
            nc.sync.dma_start(Bv[:], dv)
            BIG = pool.tile([128, k1 + k2 + 2 * w], F16, tag="BIG")
            nc.scalar.dma_start(BIG[:, k1:k1 + k2 + w], df1)
            nc.sync.dma_start(BIG[:, k1 + k2 + w:], df2)

            # constants
            wb = pool.tile([128, 1], F32, tag="wb")
            nc.vector.memset(wb[:], 0.0)
            bias1 = pool.tile([128, 1], F32, tag="bias1")
            nc.vector.memset(bias1[:], 1.0 + 1e-6)
            ones_t = pool.tile([128, 1], F32, tag="ones_t")
            nc.vector.memset(ones_t[:], 1.0)
            ones_sc = pool.tile([128, 1], F32, tag="ones_sc")
            nc.vector.memset(ones_sc[:], SC)
            ones_n = pool.tile([128, 1], F32, tag="ones_n")
            nc.vector.memset(ones_n[:], -1.0)
            # warm the DVE scan path on dummy data
            ws = pool.tile([128, 4], F32, tag="ws")
            nc.vector.tensor_tensor_scan(
                ws[:], wb[:].to_broadcast((128, 4)), wb[:].to_broadcast((128, 4)),
                0.0, op0=ALU.mult, op1=ALU.add,
            )

            V1 = Bv[:, 0:k1]
            bmax = Bv[:, k1:k1 + 1]   # batch max, host-replicated to all rows
            VV = BIG[:, 0:w]
            V2 = BIG[:, k1:w]
            RM = BIG[:, w:2 * w]
            LM = BIG[:, 2 * w:3 * w]

            # num_graphs early (only needs Bv): rng = 100/(max(batch)+1),
            # computed per-partition so it folds into the accumulations
            ng1 = pool.tile([128, 1], F32, tag="ng1")
            nc.vector.tensor_scalar(ng1[:], bmax, 1.0, 0.01,
                                    op0=ALU.add, op1=ALU.mult)
            rng = pool.tile([128, 1], F32, tag="rng")
            nc.vector.reciprocal(rng[:], ng1[:])

            # ---- Ln writes BIG's left K1 columns, completing VV
            nc.scalar.activation(BIG[:, 0:k1], V1, ACT.Ln, scale=-1.0,
                                 bias=bias1[:])
            # sum V2^2 on ACT while DVE scans (accum feeds dsq)
            sq3 = pool.tile([128, k2], F16, tag="sq3")
            Rt2 = pool.tile([128, 1], F32, tag="Rt2")
            nc.scalar.activation(sq3[:], V2, ACT.Square, bias=wb[:],
                                 accum_out=Rt2[:])

            # ---- one combined segment scan over [log | s]
            r = pool.tile([128, w], F32, tag="r")
            nc.vector.tensor_tensor_scan(
                r[:], RM, VV, 0.0, op0=ALU.mult, op1=ALU.add
            )
            # exp half first so ACT's Exp overlaps the s-half mult
            dmc1 = pool.tile([128, k1], F32, tag="dmc1")
            nc.vector.tensor_tensor(dmc1[:], r[:, 0:k1], LM[:, 0:k1],
                                    op=ALU.mult)
            e1 = pool.tile([128, k1], F32, tag="e1")
            er = pool.tile([128, 1], F32, tag="er")
            nc.scalar.activation(e1[:], dmc1[:], ACT.Exp, bias=wb[:],
                                 accum_out=er[:])
            dmc2 = pool.tile([128, k2], F32, tag="dmc2")
            nc.vector.tensor_tensor(dmc2[:], r[:, k1:w], LM[:, k1:w],
                                    op=ALU.mult)
            # Rt1r = rng * rowsum(dmc2^2)   (rng folded into the product)
            sq2 = pool.tile([128, k2], F32, tag="sq2")
            Rt1r = pool.tile([128, 1], F32, tag="Rt1r")
            nc.vector.scalar_tensor_tensor(
                sq2[:], dmc2[:], rng[:], dmc2[:],
                op0=ALU.mult, op1=ALU.mult, accum_out=Rt1r[:],
            )
            # Rt2r = rng*Rt2 + corr*SC/128, so that the three accumulating
            # matmuls below sum to SC*(sum er - corr) + rng*(Rt1 - Rt2)
            corr = float(128 * k1 - N_NODES)
            cvec = pool.tile([128, 1], F32, tag="cvec")
            nc.vector.memset(cvec[:], corr * SC / 128.0)
            Rt2r = pool.tile([128, 1], F32, tag="Rt2r")
            nc.vector.scalar_tensor_tensor(
                Rt2r[:], Rt2[:], rng[:], cvec[:], op0=ALU.mult, op1=ALU.add,
            )

            # ---- the final combine IS the partition reduce: three 1-column
            # matmuls accumulate into one PSUM scalar, each firing as soon
            # as its operand is ready (ordered by expected ready time)
            F = ppool.tile([1, 1], F32, tag="F")
            nc.tensor.matmul(F[:], ones_n[:], Rt2r[:], start=True, stop=False,
                             skip_group_check=True)
            nc.tensor.matmul(F[:], ones_sc[:], er[:], start=False, stop=False,
                             skip_group_check=True)
            nc.tensor.matmul(F[:], ones_t[:], Rt1r[:], start=False, stop=True,
                             skip_group_check=True)
            cp0 = pool.tile([1, 1], F32, tag="cp0")
            nc.vector.tensor_copy(cp0[:], F[:])
            nc.sync.dma_start(outd, cp0[:])

    nc.compile()
    return nc


def _pack_list(nodes, vals, K):
    """Sort (node, value) pairs, bin node n into partition n // 32, emit
    aligned V / RM / LM [128, K] arrays.  Index work + reordering only."""
    order = np.argsort(nodes, kind="stable")
    nodes = nodes[order]
    vals = vals[order]
    blk = nodes // NPP
    starts = np.searchsorted(blk, np.arange(128), "left")
    cnt = np.bincount(blk, minlength=128)
    if cnt.max() > K:
        return None
    pos = np.arange(len(nodes)) - starts[blk]

    V = np.zeros((128, K), np.float32)
    RM = np.ones((128, K), np.float32)
    LM = np.zeros((128, K), np.float32)
    V[blk, pos] = vals
    first = np.ones(len(nodes), bool)
    first[1:] = nodes[1:] != nodes[:-1]
    RM[blk, pos] = (~first).astype(np.float32)
    last = np.ones(len(nodes), bool)
    last[:-1] = nodes[1:] != nodes[:-1]
    LM[blk, pos] = last.astype(np.float32)
    return V, RM, LM


_CACHE = {}


def _get(key, builder, *a):
    if key not in _CACHE:
        _CACHE[key] = builder(*a)
    return _CACHE[key]


def kernel(x, edge_index, edge_feature, batch, _trace=False):
    ei = np.asarray(edge_index).astype(np.int64)
    p = np.asarray(edge_feature).astype(np.float32)[:, 0]
    batch = np.asarray(batch).astype(np.int64)
    uu = ei[0]
    tt = ei[1]

    # log list: every edge scatters at its target
    # s list: every edge at its target + non-self-loop edges at their source
    nsl = uu != tt
    nodes2 = np.concatenate([tt, uu[nsl]])
    vals2 = np.concatenate([p, p[nsl]])

    k1, k2 = K1, K2
    while True:
        p1 = _pack_list(tt, p, k1)
        if p1 is not None:
            break
        k1 += 32
    while True:
        p2 = _pack_list(nodes2, vals2, k2)
        if p2 is not None:
            break
        k2 += 32

    nc = _get((k1, k2), _build, k1, k2)

    misc = np.full((128, 1), float(batch.max()), np.float32)
    dinv = np.concatenate([p1[0], misc], axis=1).astype(np.float16)
    dinf1 = np.concatenate(
        [p2[0], p1[1], p2[1]], axis=1
    ).astype(np.float16)   # V2 | RM1 | RM2
    dinf2 = np.concatenate([p1[2], p2[2]], axis=1).astype(np.float16)

    r = bass_utils.run_bass_kernel_spmd(
        nc, [{"dinv": dinv, "dinf1": dinf1, "dinf2": dinf2}],
        core_ids=[0], trace=_trace,
    )
    out = np.asarray(r.results[0]["out"], dtype=np.float32).reshape(1, 1)
    if _trace:
        kernel.last_results = (r,)
    return out


# revision 28
# speedup vs baseline: 1.0029x; 1.0029x over previous
"""Trainium2 Bass kernel for nn_ErdosLoss (graph loss function).

Math (reference reformulated, validated to ~1e-6 rel err):
  penalty:  log_score = scatter_add(log(1 - p + 1e-6), tgt)   over N nodes
            loss2 = mean(exp(log_score)) * 9600
  loss3:    p @ triu(H H^T, 1) @ p^T  ==  (||s||^2 - sum_e d_e p_e^2) / 2
            where s = scatter_add(p, tgt) + scatter_add(p, src | src != tgt),
            d_e = 2 - m_e, m_e = (src_e == tgt_e).
  out = loss2 + 200 * loss3 / num_graphs,  num_graphs = max(batch) + 1.

Single-launch single-core design (per-NEFF fixed overhead here is ~11us, so
any second launch loses):
  Host (index-only preprocessing; values are only reordered, never
  combined): sort the (node, value) scatter pairs by node, bin them so that
  partition p holds exactly nodes [32p, 32(p+1)), and emit aligned [128, K]
  arrays per list: V (values), RM (0 at each node's first pair, else 1),
  LM (1 at each node's last pair, else 0).

  Device: both lists ride ONE segment scan.  A single f16 tile BIG holds
  [Ln-region(K1) | V2(K2) | RM(K1+K2) | LM(K1+K2)]: ACT's Ln writes the
  left K1 columns in place, one f16 DMA fills the rest, so the combined
  tensor_tensor_scan (state = RM*state + v, resetting at segment starts)
  covers the log list and the s list back to back (each partition's s
  section starts with rm=0, so no cross-contamination).  d = r*LM isolates
  node totals; exp over the log half row-accumulates (masked slots count
  exp(0)=1 -> compile-time correction 128*K1-4000; empty nodes cancel);
  the s half yields sum(d^2) - sum(V2^2) via one accumulated product and
  an ACT Square.  R = [SC*er - SC*corr/128 | dsq]; ones-matmul partition
  reduce to PSUM [1,2]; res = F1*(100/num_graphs) + F0; one [1,1] DMA out.

  Latency layout: one explicit ACT table load (set 6 = Ln+Exp+Square+Copy,
  so no mid-kernel reloads); exactly two f16 input transfers (V1+misc on
  sync, the [V2|RM|LM] block on scalar) since a queue's second transfer
  pays a ~0.7us re-arm gap; the exp mask-mult is split off first so ACT's
  Exp overlaps the s-half mask-mult; num_graphs is host-replicated across
  partitions so 100/num_graphs folds into the per-partition accumulations
  and the ones-matmul emits the final scalar directly.
"""

import numpy as np

import concourse.bacc as bacc
import concourse.mybir as mybir
import concourse.tile as tile
from concourse import bass_utils

F32 = mybir.dt.float32
F16 = mybir.dt.float16
ALU = mybir.AluOpType
ACT = mybir.ActivationFunctionType

N_NODES = 4000
PENALTY_SCALE = 16 * 200 * 3   # 9600
SC = PENALTY_SCALE / N_NODES   # 2.4
NPP = 32                       # nodes per partition (128 * 32 = 4096 >= 4000)

K1 = 64    # log-list slots per partition  (measured max 63)
K2 = 120   # s-list slots per partition    (measured max 120; exact fit --
           # _pack_list falls back to a wider rebuild if ever exceeded)


def _build(k1: int, k2: int):
    nc = bacc.Bacc("TRN2", target_bir_lowering=False, debug=False, num_devices=1)

    w = k1 + k2
    dv = nc.dram_tensor("dinv", [128, k1 + 1], F16, kind="ExternalInput").ap()
    df = nc.dram_tensor("dinf", [128, k2 + 2 * w], F16, kind="ExternalInput").ap()
    outd = nc.dram_tensor("out", [1, 1], F32, kind="ExternalOutput").ap()

    with tile.TileContext(nc) as tc:
        with (
            tc.tile_pool(name="pool", bufs=1) as pool,
            tc.tile_pool(name="psum", bufs=1, space="PSUM") as ppool,
        ):
            # one ACT table set covering Ln+Exp+Square+Copy (set 6); the
            # auto placement pass then inserts no further loads
            nc.scalar.add_instruction(mybir.InstLoadActFuncSet(
                name="actload6", ins=[], outs=[], act_func_set_id=6))

            # two input transfers, one per HWDGE queue (a second transfer on
            # the same queue pays a ~0.7us DGE re-arm gap)
            Bv = pool.tile([128, k1 + 1], F16, tag="Bv")
            nc.sync.dma_start(Bv[:], dv)
            BIG = pool.tile([128, k1 + k2 + 2 * w], F16, tag="BIG")
            nc.scalar.dma_start(BIG[:, k1:], df)

            # constants
            wb = pool.tile([128, 1], F32, tag="wb")
            nc.vector.memset(wb[:], 0.0)
            bias1 = pool.tile([128, 1], F32, tag="bias1")
            nc.vector.memset(bias1[:], 1.0 + 1e-6)
            ones_t = pool.tile([128, 1], F32, tag="ones_t")
            nc.vector.memset(ones_t[:], 1.0)
            ones_sc = pool.tile([128, 1], F32, tag="ones_sc")
            nc.vector.memset(ones_sc[:], SC)
            ones_n = pool.tile([128, 1], F32, tag="ones_n")
            nc.vector.memset(ones_n[:], -1.0)
            # warm the DVE scan path on dummy data
            ws = pool.tile([128, 4], F32, tag="ws")
            nc.vector.tensor_tensor_scan(
                ws[:], wb[:].to_broadcast((128, 4)), wb[:].to_broadcast((128, 4)),
                0.0, op0=ALU.mult, op1=ALU.add,
            )

            V1 = Bv[:, 0:k1]
            bmax = Bv[:, k1:k1 + 1]   # batch max, host-replicated to all rows
            VV = BIG[:, 0:w]
            V2 = BIG[:, k1:w]
            RM = BIG[:, w:2 * w]
            LM = BIG[:, 2 * w:3 * w]

            # num_graphs early (only needs Bv): rng = 100/(max(batch)+1),
            # computed per-partition so it folds into the accumulations
            ng1 = pool.tile([128, 1], F32, tag="ng1")
            nc.vector.tensor_scalar(ng1[:], bmax, 1.0, 0.01,
                                    op0=ALU.add, op1=ALU.mult)
            rng = pool.tile([128, 1], F32, tag="rng")
            nc.vector.reciprocal(rng[:], ng1[:])

            # ---- Ln writes BIG's left K1 columns, completing VV
            nc.scalar.activation(BIG[:, 0:k1], V1, ACT.Ln, scale=-1.0,
                                 bias=bias1[:])
            # sum V2^2 on ACT while DVE scans (accum feeds dsq)
            sq3 = pool.tile([128, k2], F16, tag="sq3")
            Rt2 = pool.tile([128, 1], F32, tag="Rt2")
            nc.scalar.activation(sq3[:], V2, ACT.Square, bias=wb[:],
                                 accum_out=Rt2[:])

            # ---- one combined segment scan over [log | s]
            r = pool.tile([128, w], F32, tag="r")
            nc.vector.tensor_tensor_scan(
                r[:], RM, VV, 0.0, op0=ALU.mult, op1=ALU.add
            )
            # exp half first so ACT's Exp overlaps the s-half mult
            dmc1 = pool.tile([128, k1], F32, tag="dmc1")
            nc.vector.tensor_tensor(dmc1[:], r[:, 0:k1], LM[:, 0:k1],
                                    op=ALU.mult)
            e1 = pool.tile([128, k1], F32, tag="e1")
            er = pool.tile([128, 1], F32, tag="er")
            nc.scalar.activation(e1[:], dmc1[:], ACT.Exp, bias=wb[:],
                                 accum_out=er[:])
            dmc2 = pool.tile([128, k2], F32, tag="dmc2")
            nc.vector.tensor_tensor(dmc2[:], r[:, k1:w], LM[:, k1:w],
                                    op=ALU.mult)
            # Rt1r = rng * rowsum(dmc2^2)   (rng folded into the product)
            sq2 = pool.tile([128, k2], F32, tag="sq2")
            Rt1r = pool.tile([128, 1], F32, tag="Rt1r")
            nc.vector.scalar_tensor_tensor(
                sq2[:], dmc2[:], rng[:], dmc2[:],
                op0=ALU.mult, op1=ALU.mult, accum_out=Rt1r[:],
            )
            # Rt2r = rng*Rt2 + corr*SC/128, so that the three accumulating
            # matmuls below sum to SC*(sum er - corr) + rng*(Rt1 - Rt2)
            corr = float(128 * k1 - N_NODES)
            cvec = pool.tile([128, 1], F32, tag="cvec")
            nc.vector.memset(cvec[:], corr * SC / 128.0)
            Rt2r = pool.tile([128, 1], F32, tag="Rt2r")
            nc.vector.scalar_tensor_tensor(
                Rt2r[:], Rt2[:], rng[:], cvec[:], op0=ALU.mult, op1=ALU.add,
            )

            # ---- the final combine IS the partition reduce: three 1-column
            # matmuls accumulate into one PSUM scalar, each firing as soon
            # as its operand is ready (ordered by expected ready time)
            F = ppool.tile([1, 1], F32, tag="F")
            nc.tensor.matmul(F[:], ones_n[:], Rt2r[:], start=True, stop=False,
                             skip_group_check=True)
            nc.tensor.matmul(F[:], ones_sc[:], er[:], start=False, stop=False,
                             skip_group_check=True)
            nc.tensor.matmul(F[:], ones_t[:], Rt1r[:], start=False, stop=True,
                             skip_group_check=True)
            cp0 = pool.tile([1, 1], F32, tag="cp0")
            nc.vector.tensor_copy(cp0[:], F[:])
            nc.sync.dma_start(outd, cp0[:])

    nc.compile()
    return nc


def _pack_list(nodes, vals, K):
    """Sort (node, value) pairs, bin node n into partition n // 32, emit
    aligned V / RM / LM [128, K] arrays.  Index work + reordering only."""
    order = np.argsort(nodes, kind="stable")
    nodes = nodes[order]
    vals = vals[order]
    blk = nodes // NPP
    starts = np.searchsorted(blk, np.arange(128), "left")
    cnt = np.bincount(blk, minlength=128)
    if cnt.max() > K:
        return None
    pos = np.arange(len(nodes)) - starts[blk]

    V = np.zeros((128, K), np.float32)
    RM = np.ones((128, K), np.float32)
    LM = np.zeros((128, K), np.float32)
    V[blk, pos] = vals
    first = np.ones(len(nodes), bool)
    first[1:] = nodes[1:] != nodes[:-1]
    RM[blk, pos] = (~first).astype(np.float32)
    last = np.ones(len(nodes), bool)
    last[:-1] = nodes[1:] != nodes[:-1]
    LM[blk, pos] = last.astype(np.float32)
    return V, RM, LM


_CACHE = {}


def _get(key, builder, *a):
    if key not in _CACHE:
        _CACHE[key] = builder(*a)
    return _CACHE[key]


def kernel(x, edge_index, edge_feature, batch, _trace=False):
    ei = np.asarray(edge_index).astype(np.int64)
    p = np.asarray(edge_feature).astype(np.float32)[:, 0]
    batch = np.asarray(batch).astype(np.int64)
    uu = ei[0]
    tt = ei[1]

    # log list: every edge scatters at its target
    # s list: every edge at its target + non-self-loop edges at their source
    nsl = uu != tt
    nodes2 = np.concatenate([tt, uu[nsl]])
    vals2 = np.concatenate([p, p[nsl]])

    k1, k2 = K1, K2
    while True:
        p1 = _pack_list(tt, p, k1)
        if p1 is not None:
            break
        k1 += 32
    while True:
        p2 = _pack_list(nodes2, vals2, k2)
        if p2 is not None:
            break
        k2 += 32

    nc = _get((k1, k2), _build, k1, k2)

    misc = np.full((128, 1), float(batch.max()), np.float32)
    dinv = np.concatenate([p1[0], misc], axis=1).astype(np.float16)
    dinf = np.concatenate(
        [p2[0], p1[1], p2[1], p1[2], p2[2]], axis=1
    ).astype(np.float16)   # V2 | RM1 | RM2 | LM1 | LM2

    r = bass_utils.run_bass_kernel_spmd(
        nc, [{"dinv": dinv, "dinf": dinf}], core_ids=[0], trace=_trace,
    )
    out = np.asarray(r.results[0]["out"], dtype=np.float32).reshape(1, 1)
    if _trace:
        kernel.last_results = (r,)
    return out
